# revision 1
# baseline (speedup 1.0000x reference)
"""MLA-style sparse-attention GPT block on 8 Trainium2 NeuronCores.

Sharding: tensor-parallel over heads x data-parallel over batch.
Core c handles batch b = c // 4 and heads [4*hg, 4*hg+4) with hg = c % 4.
Each core computes its partial c_proj output (2048, 1024); the host sums
the 4 partials per batch.

Layout convention on-device: activations are stored transposed
(features on partitions, T on the free dim), so x is fed in as
xT = x[b].T. RoPE is folded into the up-projection matmuls via a
host-precomputed signed-permutation matrix; causal softmax is computed
in scoresT layout (keys on partitions) so the denominator comes for
free from a ones-augmented V matmul.
"""

import sys

sys.path.insert(0, "/opt/trn_rl_repo")

import ml_dtypes
import numpy as np

import concourse.bass as bass
import concourse.tile as tile
from concourse import bacc
from concourse import mybir
from concourse.bass_utils import run_bass_kernel_spmd

B, T, C = 2, 2048, 1024
H, L = 16, 64
DH = 64
DHE = 32
THETA = 10000.0

HG = 4  # head-groups (cores per batch)
HPG = H // HG  # heads per core = 4
FT = HPG // 2  # "final tiles" per core: 2 heads each -> 2 tiles of 128 rows

KC = C // 128  # 8 contraction chunks for the down-projection
TC = T // 512  # 4 chunks of 512 along T
QB = T // 512  # query chunks of 512
KB = T // 128  # key blocks of 128

F32 = mybir.dt.float32
BF16 = mybir.dt.bfloat16
F32R = mybir.dt.float32r

_NC_CACHE = {}


def _build_nc():
    if "nc" in _NC_CACHE:
        return _NC_CACHE["nc"]
    nc = bacc.Bacc("TRN2", target_bir_lowering=False)

    xT = nc.dram_tensor("xT", [C, T], BF16, kind="ExternalInput")
    wqd = nc.dram_tensor("wqd", [C, HPG * L], BF16, kind="ExternalInput")
    wkd = nc.dram_tensor("wkd", [C, HPG * L], BF16, kind="ExternalInput")
    wvd = nc.dram_tensor("wvd", [C, HPG * L], BF16, kind="ExternalInput")
    ceq = nc.dram_tensor("ceq", [FT, 128, 128], BF16, kind="ExternalInput")
    rotq = nc.dram_tensor("rotq", [FT, 128, 128], BF16, kind="ExternalInput")
    cek = nc.dram_tensor("cek", [FT, 128, 128], BF16, kind="ExternalInput")
    rotk = nc.dram_tensor("rotk", [FT, 128, 128], BF16, kind="ExternalInput")
    vu2 = nc.dram_tensor("vu2", [128, DH], BF16, kind="ExternalInput")
    cosM = nc.dram_tensor("cosM", [128, T], F32, kind="ExternalInput")
    sinM = nc.dram_tensor("sinM", [128, T], F32, kind="ExternalInput")
    mask4 = nc.dram_tensor("mask4", [128, 4 * 512], BF16, kind="ExternalInput")
    wcs = nc.dram_tensor("wcs", [HPG * L, C], BF16, kind="ExternalInput")
    out = nc.dram_tensor("out", [T, C], F32, kind="ExternalOutput")

    with tile.TileContext(nc) as tc:
        _emit(nc, tc, xT, wqd, wkd, wvd, ceq, rotq, cek, rotk, vu2,
              cosM, sinM, mask4, wcs, out)
    nc.compile()

    _NC_CACHE["nc"] = nc
    return nc


def _emit(nc, tc, xT, wqd, wkd, wvd, ceq, rotq, cek, rotk, vu2,
          cosM, sinM, mask4, wcs, out):
    from contextlib import ExitStack

    ctx = ExitStack()
    with ctx:
        consts = ctx.enter_context(tc.tile_pool(name="consts", bufs=1))
        persist = ctx.enter_context(tc.tile_pool(name="persist", bufs=1))

        # ---- constants that live for the whole kernel ----
        vu2_sb = consts.tile([128, DH], BF16, tag="vu2", name="vu2")
        nc.sync.dma_start(vu2_sb, vu2[:, :])
        mask_sb = consts.tile([128, 4 * 512], BF16, tag="mask", name="mask")
        nc.sync.dma_start(mask_sb, mask4[:, :])
        wcs_sb = [consts.tile([64, C], BF16, tag=f"wcs{t}", name=f"wcs{t}") for t in range(2 * FT)]
        for t in range(2 * FT):
            nc.sync.dma_start(wcs_sb[t], wcs[t * 64:(t + 1) * 64, :])

        # ---- persistent activations ----
        qfin = [persist.tile([128, T], BF16, tag=f"qfin{t}", name=f"qfin{t}") for t in range(FT)]
        kfin = [persist.tile([128, T], BF16, tag=f"kfin{t}", name=f"kfin{t}") for t in range(FT)]
        vlat = [persist.tile([128, T], BF16, tag=f"vlat{t}", name=f"vlat{t}") for t in range(FT)]
        ycore = [persist.tile([64, T], BF16, tag=f"ycore{t}", name=f"ycore{t}") for t in range(2 * FT)]

        # ================= projection phase =================
        with tc.tile_pool(name="proj_w", bufs=1) as pw, \
             tc.tile_pool(name="proj_ps", bufs=1, space="PSUM") as pps, \
             tc.tile_pool(name="proj_up_ps", bufs=1, space="PSUM") as ups, \
             tc.tile_pool(name="proj_sb", bufs=2) as psb, \
             tc.tile_pool(name="xpieces", bufs=33) as xpool:
            # projection-phase-only constants
            dwq = [pw.tile([128, HPG * L], BF16, tag=f"dwq{k}", name=f"dwq{k}") for k in range(KC)]
            dwk = [pw.tile([128, HPG * L], BF16, tag=f"dwk{k}", name=f"dwk{k}") for k in range(KC)]
            dwv = [pw.tile([128, HPG * L], BF16, tag=f"dwv{k}", name=f"dwv{k}") for k in range(KC)]
            for k in range(KC):
                nc.sync.dma_start(dwq[k], wqd[k * 128:(k + 1) * 128, :])
                nc.sync.dma_start(dwk[k], wkd[k * 128:(k + 1) * 128, :])
                nc.sync.dma_start(dwv[k], wvd[k * 128:(k + 1) * 128, :])
            upw = {}
            for name, src in (("ceq", ceq), ("rotq", rotq),
                              ("cek", cek), ("rotk", rotk)):
                upw[name] = [pw.tile([128, 128], BF16, tag=f"{name}{t}", name=f"{name}{t}")
                             for t in range(FT)]
                for t in range(FT):
                    nc.sync.dma_start(upw[name][t], src[t, :, :])
            cos_sb = pw.tile([128, T], F32, tag="cos", name="cos")
            sin_sb = pw.tile([128, T], F32, tag="sin", name="sin")
            nc.sync.dma_start(cos_sb, cosM[:, :])
            nc.sync.dma_start(sin_sb, sinM[:, :])
            for t in range(TC):
                tsl = slice(t * 512, (t + 1) * 512)
                # down-projection: 6 accumulating banks (q0 q1 k0 k1 v0 v1)
                lat_ps = [pps.tile([128, 512], F32, tag=f"lat{i}", name=f"lat{i}") for i in range(6)]
                for k in range(KC):
                    xp = xpool.tile([128, 512], BF16, tag="xp", name="xp")
                    nc.sync.dma_start(xp, xT[k * 128:(k + 1) * 128, tsl])
                    for w, base in ((dwq, 0), (dwk, 2), (dwv, 4)):
                        for ft in range(FT):
                            nc.tensor.matmul(
                                lat_ps[base + ft],
                                lhsT=(w[k][:, ft * 128:(ft + 1) * 128]),
                                rhs=(xp),
                                start=(k == 0), stop=(k == KC - 1))
                lat_sb = [psb.tile([128, 512], BF16, tag=f"latsb{i}", name=f"latsb{i}") for i in range(4)]
                for i in range(4):
                    nc.scalar.copy(lat_sb[i], lat_ps[i])
                for ft in range(FT):
                    nc.scalar.copy(vlat[ft][:, tsl], lat_ps[4 + ft])
                # up-projection + rope for q and k
                for fin, lats, cew, rotw in ((qfin, lat_sb[0:2], upw["ceq"], upw["rotq"]),
                                             (kfin, lat_sb[2:4], upw["cek"], upw["rotk"])):
                    for ft in range(FT):
                        cep = ups.tile([128, 512], F32, tag="cep", name="cep")
                        nc.tensor.matmul(cep, lhsT=(cew[ft]), rhs=(lats[ft]),
                                         start=True, stop=True)
                        rop = ups.tile([128, 512], F32, tag="rop", name="rop")
                        nc.tensor.matmul(rop, lhsT=(rotw[ft]), rhs=(lats[ft]),
                                         start=True, stop=True)
                        tmp1 = psb.tile([128, 512], F32, tag="tmp1", name="tmp1")
                        tmp2 = psb.tile([128, 512], F32, tag="tmp2", name="tmp2")
                        nc.vector.tensor_mul(tmp1, cep, cos_sb[:, tsl])
                        nc.vector.tensor_mul(tmp2, rop, sin_sb[:, tsl])
                        nc.vector.tensor_add(fin[ft][:, tsl], tmp1, tmp2)

        # ================= attention phase =================
        with tc.tile_pool(name="sc_ps", bufs=3, space="PSUM") as scp, \
             tc.tile_pool(name="yt_ps", bufs=4, space="PSUM") as ytp, \
             tc.tile_pool(name="vn_ps", bufs=1, space="PSUM") as vnp, \
             tc.tile_pool(name="att_sb", bufs=4) as asb, \
             tc.tile_pool(name="vaug_sb", bufs=2) as vsb, \
             tc.tile_pool(name="dram_scr", bufs=2, space="DRAM") as dsp, \
             tc.tile_pool(name="small_sb", bufs=4) as ssb:
            for ft in range(FT):
                for off in (0, 64):
                    hsl = slice(off, off + 64)
                    h = 2 * ft + (1 if off else 0)
                    # v in natural layout (keys on partitions) + ones column
                    vaug = vsb.tile([128, KB * (DH + 1)], BF16, tag="vaug", name="vaug")
                    va3 = vaug.rearrange("p (b c) -> p b c", c=DH + 1)
                    nc.vector.memset(va3[:, :, DH], 1.0)
                    for blk in range(KB):
                        vp = vnp.tile([128, DH], F32, tag="vn", name="vn")
                        nc.tensor.matmul(
                            vp,
                            lhsT=(vlat[ft][hsl, blk * 128:(blk + 1) * 128]),
                            rhs=(vu2_sb[hsl, :]),
                            start=True, stop=True)
                        nc.scalar.copy(vaug[:, blk * 65:blk * 65 + DH], vp)
                    # causal attention, scoresT layout
                    den = ssb.tile([65, T], F32, tag="den", name="den")
                    yps = []
                    for j in range(QB):
                        qsl = slice(j * 512, (j + 1) * 512)
                        yp = ytp.tile([DH + 1, 512], F32, tag="yt", name="yt")
                        yps.append(yp)
                        nblk = 4 * j + 4
                        for i in range(nblk):
                            sp = scp.tile([128, 512], F32, tag="sc", name="sc")
                            nc.tensor.matmul(
                                sp,
                                lhsT=(kfin[ft][hsl, i * 128:(i + 1) * 128]),
                                rhs=(qfin[ft][hsl, qsl]),
                                start=True, stop=True)
                            pr = asb.tile([128, 512], BF16, tag="pr", name="pr")
                            nc.scalar.activation(pr, sp, mybir.ActivationFunctionType.Exp)
                            d = i - 4 * j
                            if d >= 0:
                                nc.vector.tensor_mul(
                                    pr, pr, mask_sb[:, d * 512:(d + 1) * 512])
                            nc.tensor.matmul(
                                yp, lhsT=(vaug[:, i * 65:(i + 1) * 65]), rhs=(pr),
                                start=(i == 0), stop=(i == nblk - 1))
                        nc.vector.tensor_copy(
                            den[DH:DH + 1, qsl], yp[DH:DH + 1, :])
                    # one reciprocal + broadcast, then normalize straight
                    # out of the still-live PSUM tiles
                    nc.vector.reciprocal(den[DH:DH + 1, :], den[DH:DH + 1, :])
                    rec_d = dsp.tile([1, T], F32, tag="rec_d", name="rec_d")
                    nc.sync.dma_start(rec_d, den[DH:DH + 1, :])
                    rec64 = ssb.tile([64, T], F32, tag="rec64", name="rec64")
                    nc.sync.dma_start(
                        rec64,
                        bass.AP(tensor=rec_d.tensor, offset=rec_d.offset,
                                ap=[[0, 64], [1, T]]))
                    for j in range(QB):
                        qsl = slice(j * 512, (j + 1) * 512)
                        nc.vector.tensor_mul(
                            ycore[h][:, qsl], yps[j][0:DH, :], rec64[:, qsl])

        # ================= output projection =================
        with tc.tile_pool(name="out_ps", bufs=4, space="PSUM") as ops, \
             tc.tile_pool(name="out_sb", bufs=4) as osbp:
            for m in range(T // 128):
                msl = slice(m * 128, (m + 1) * 128)
                for n in range(C // 512):
                    op = ops.tile([128, 512], F32, tag="op", name="op")
                    for kt in range(2 * FT):
                        nc.tensor.matmul(
                            op,
                            lhsT=(ycore[kt][:, msl]),
                            rhs=(wcs_sb[kt][:, n * 512:(n + 1) * 512]),
                            start=(kt == 0), stop=(kt == 2 * FT - 1))
                    osb = osbp.tile([128, 512], F32, tag="osb", name="osb")
                    nc.scalar.copy(osb, op)
                    nc.sync.dma_start(out[msl, n * 512:(n + 1) * 512], osb)


def _host_prep(x, Wq_down, Wk_down, Wv_down, Wq_up_c, Wq_up_e, Wk_up_c,
               Wk_up_e, Wv_up, Wc):
    """Build the per-core input maps."""
    import math

    scale = 1.0 / math.sqrt(DH)

    # rope cache, transposed: (DHE, T)
    inv_freq = 1.0 / (THETA ** (np.arange(0, DHE, 2, dtype=np.float32) / DHE))
    freqs = np.arange(T, dtype=np.float32)[:, None] * inv_freq[None, :]
    emb = np.concatenate((freqs, freqs), axis=-1)  # (T, 32)
    cosT = np.cos(emb).T.astype(np.float32)  # (32, T)
    sinT = np.sin(emb).T.astype(np.float32)

    # signed permutation P: rot = P @ x with rot[2i] = -x[2i+1], rot[2i+1] = x[2i]
    P = np.zeros((DHE, DHE), dtype=np.float32)
    for i in range(DHE // 2):
        P[2 * i, 2 * i + 1] = -1.0
        P[2 * i + 1, 2 * i] = 1.0

    def ce_lhsT(Wc_, We_, s):
        # (128, 128): latents of 2 heads on partitions ->
        # [c_even | e_even | c_odd | e_odd] output rows
        m = np.zeros((128, 128), dtype=np.float32)
        m[0:64, 0:32] = Wc_ * s
        m[0:64, 32:64] = We_ * s
        m[64:128, 64:96] = Wc_ * s
        m[64:128, 96:128] = We_ * s
        return m

    def rot_lhsT(We_, s):
        m = np.zeros((128, 128), dtype=np.float32)
        wr = (We_ @ P.T) * s
        m[0:64, 32:64] = wr
        m[64:128, 96:128] = wr
        return m

    # identical for both final tiles -> replicate
    ceq = np.stack([ce_lhsT(Wq_up_c, Wq_up_e, scale)] * FT)
    rotq = np.stack([rot_lhsT(Wq_up_e, scale)] * FT)
    cek = np.stack([ce_lhsT(Wk_up_c, Wk_up_e, 1.0)] * FT)
    rotk = np.stack([rot_lhsT(Wk_up_e, 1.0)] * FT)
    vu2 = np.concatenate([Wv_up, Wv_up], axis=0).astype(np.float32)  # (128, 64)

    # cosM rows: [ones, cosT, ones, cosT]; sinM rows: [0, sinT, 0, sinT]
    ones = np.ones((32, T), dtype=np.float32)
    zeros = np.zeros((32, T), dtype=np.float32)
    cosM = np.concatenate([ones, cosT, ones, cosT], axis=0)
    sinM = np.concatenate([zeros, sinT, zeros, sinT], axis=0)

    # mask variants d=0..3: allowed iff kk <= qq - 128*d
    kk = np.arange(128)[:, None]
    qq = np.arange(512)[None, :]
    mask4 = np.concatenate(
        [(kk <= qq - 128 * d).astype(np.float32) for d in range(4)], axis=1)

    xTs = [np.ascontiguousarray(x[b].T).astype(np.float32) for b in range(B)]

    bf = ml_dtypes.bfloat16
    in_maps = []
    for core in range(8):
        b, hg = core // HG, core % HG
        csl = slice(hg * HPG * L, (hg + 1) * HPG * L)
        in_maps.append({
            "xT": xTs[b].astype(bf),
            "wqd": np.ascontiguousarray(Wq_down[:, csl]).astype(bf),
            "wkd": np.ascontiguousarray(Wk_down[:, csl]).astype(bf),
            "wvd": np.ascontiguousarray(Wv_down[:, csl]).astype(bf),
            "ceq": ceq.astype(bf), "rotq": rotq.astype(bf),
            "cek": cek.astype(bf), "rotk": rotk.astype(bf),
            "vu2": vu2.astype(bf), "cosM": cosM, "sinM": sinM,
            "mask4": mask4.astype(bf),
            "wcs": np.ascontiguousarray(Wc[csl, :]).astype(bf),
        })
    return in_maps


LAST_RESULT = {}


def kernel(**inputs):
    inputs = {k: np.asarray(v, dtype=np.float32) for k, v in inputs.items()}
    nc = _build_nc()
    in_maps = _host_prep(**inputs)
    res = run_bass_kernel_spmd(nc, in_maps, core_ids=list(range(8)))
    LAST_RESULT.clear()
    LAST_RESULT.update(
        exec_time_ns=res.exec_time_ns,
        mean_exec_time_ns=res.mean_exec_time_ns,
        profile_json=res.profile_json,
    )
    parts = [r["out"] for r in res.results]
    out = np.stack([
        parts[0] + parts[1] + parts[2] + parts[3],
        parts[4] + parts[5] + parts[6] + parts[7],
    ])
    return out.astype(np.float32)


if __name__ == "__main__":
    rng = np.random.default_rng(0)
    ins = {
        "x": rng.standard_normal((B, T, C), dtype=np.float32),
        "Wq_down": rng.standard_normal((C, H * L), dtype=np.float32) * 0.02,
        "Wk_down": rng.standard_normal((C, H * L), dtype=np.float32) * 0.02,
        "Wv_down": rng.standard_normal((C, H * L), dtype=np.float32) * 0.02,
        "Wq_up_c": rng.standard_normal((L, DHE), dtype=np.float32) * 0.02,
        "Wq_up_e": rng.standard_normal((L, DHE), dtype=np.float32) * 0.02,
        "Wk_up_c": rng.standard_normal((L, DHE), dtype=np.float32) * 0.02,
        "Wk_up_e": rng.standard_normal((L, DHE), dtype=np.float32) * 0.02,
        "Wv_up": rng.standard_normal((L, DH), dtype=np.float32) * 0.02,
        "Wc": rng.standard_normal((C, C), dtype=np.float32) * 0.02,
    }
    y = kernel(**ins)
    print(y.shape, y.dtype, float(np.abs(y).mean()))



# revision 2
# speedup vs baseline: 11575.0890x; 11575.0890x over previous
"""MLA-style sparse-attention GPT block on 8 Trainium2 NeuronCores — v2.

Sharding: tensor-parallel over heads x data-parallel over batch.
Core c handles batch b = c // 4 and heads [4*hg, 4*hg+4) with hg = c % 4.
Each core computes its partial c_proj output (2048, 1024); the host sums
the 4 partials per batch.

v2 layout/scheduling changes vs v1:
- all inputs arrive in a handful of large DMAs (k-major packed weights),
  x is held resident in SBUF as 8 [128, 2048] tiles
- v up-projection + vaug assembly folded into the projection phase
- attention runs query-chunk-outer (j) with causal column restriction on
  the diagonal blocks; the output projection for chunk j's rows is
  interleaved right after chunk j's normalize, under the exp roof
- softmax reciprocal is a single DVE reciprocal_approx_fast on the
  denominator row; broadcast via a DRAM round-trip DMA
- ycore is packed as two [128, T] head-pair tiles so c_proj runs K=128
"""

import sys

sys.path.insert(0, "/opt/trn_rl_repo")

import ml_dtypes
import numpy as np

import concourse.bass as bass
import concourse.tile as tile
from concourse import bacc
from concourse import mybir
from concourse.bass_utils import run_bass_kernel_spmd

B, T, C = 2, 2048, 1024
H, L = 16, 64
DH = 64
DHE = 32
THETA = 10000.0

HG = 4  # head-groups (cores per batch)
HPG = H // HG  # heads per core = 4
FT = HPG // 2  # 2 latent tiles of 128 rows (2 heads each)

KC = C // 128  # 8 contraction chunks for the down-projection
TC = T // 512  # 4 chunks of 512 along T
QB = T // 512  # query chunks of 512
KB = T // 128  # key blocks of 128

F32 = mybir.dt.float32
BF16 = mybir.dt.bfloat16

USE_FAST_RECIP = True

_NC_CACHE = {}


def _build_nc():
    if "nc" in _NC_CACHE:
        return _NC_CACHE["nc"]
    nc = bacc.Bacc("TRN2", target_bir_lowering=False)

    xT = nc.dram_tensor("xT", [C, T], BF16, kind="ExternalInput")
    dwq = nc.dram_tensor("dwq", [128, KC * HPG * L // 1], BF16, kind="ExternalInput")
    dwk = nc.dram_tensor("dwk", [128, KC * HPG * L // 1], BF16, kind="ExternalInput")
    dwv = nc.dram_tensor("dwv", [128, KC * HPG * L // 1], BF16, kind="ExternalInput")
    upw = nc.dram_tensor("upw", [128, 512], BF16, kind="ExternalInput")
    vu2 = nc.dram_tensor("vu2", [128, DH], BF16, kind="ExternalInput")
    cosM = nc.dram_tensor("cosM", [128, T], F32, kind="ExternalInput")
    sinM = nc.dram_tensor("sinM", [128, T], F32, kind="ExternalInput")
    mask128 = nc.dram_tensor("mask128", [128, 128], BF16, kind="ExternalInput")
    wcs2 = nc.dram_tensor("wcs2", [2, 128, C], BF16, kind="ExternalInput")
    out = nc.dram_tensor("out", [T, C], BF16, kind="ExternalOutput")

    with tile.TileContext(nc) as tc:
        _emit(nc, tc, xT, dwq, dwk, dwv, upw, vu2, cosM, sinM, mask128,
              wcs2, out)
    nc.compile()

    _NC_CACHE["nc"] = nc
    return nc


def _emit(nc, tc, xT, dwq, dwk, dwv, upw, vu2, cosM, sinM, mask128, wcs2, out):
    from contextlib import ExitStack

    ctx = ExitStack()
    with ctx:
        consts = ctx.enter_context(tc.tile_pool(name="consts", bufs=1))
        persist = ctx.enter_context(tc.tile_pool(name="persist", bufs=1))

        # ---- batched input DMAs (order matters: first-needed first) ----
        # x held as two half-T tiles per k-chunk so chunk t<2 compute can
        # start before the back half of x arrives; all issues on the SP ring
        dwq_sb = consts.tile([128, KC * 256], BF16, tag="dwq", name="dwq")
        nc.sync.dma_start(dwq_sb, dwq[:, :])
        xA = [consts.tile([128, 1024], BF16, tag=f"xA{k}", name=f"xA{k}")
              for k in range(KC)]
        xB = [consts.tile([128, 1024], BF16, tag=f"xB{k}", name=f"xB{k}")
              for k in range(KC)]
        nc.sync.dma_start(xA[0], xT[0:128, 0:1024])
        dwk_sb = consts.tile([128, KC * 256], BF16, tag="dwk", name="dwk")
        nc.sync.dma_start(dwk_sb, dwk[:, :])
        dwv_sb = consts.tile([128, KC * 256], BF16, tag="dwv", name="dwv")
        nc.sync.dma_start(dwv_sb, dwv[:, :])
        dw_sb = [dwq_sb, dwk_sb, dwv_sb]
        vu2_sb = consts.tile([128, DH], BF16, tag="vu2", name="vu2")
        nc.sync.dma_start(vu2_sb, vu2[:, :])
        for k in (1, 2):
            nc.sync.dma_start(xA[k], xT[k * 128:(k + 1) * 128, 0:1024])
        upw_sb = consts.tile([128, 512], BF16, tag="upw", name="upw")
        nc.sync.dma_start(upw_sb, upw[:, :])
        cos_sb = consts.tile([128, T], F32, tag="cos", name="cos")
        sin_sb = consts.tile([128, T], F32, tag="sin", name="sin")
        nc.sync.dma_start(cos_sb, cosM[:, :])
        nc.sync.dma_start(sin_sb, sinM[:, :])
        for k in range(3, KC):
            nc.sync.dma_start(xA[k], xT[k * 128:(k + 1) * 128, 0:1024])
        mask_sb = consts.tile([128, 128], BF16, tag="mask", name="mask")
        nc.sync.dma_start(mask_sb, mask128[:, :])
        for k in range(KC):
            nc.sync.dma_start(xB[k], xT[k * 128:(k + 1) * 128, 1024:2048])
        wcs_sb = [consts.tile([128, C], BF16, tag=f"wcs{g}", name=f"wcs{g}")
                  for g in range(2)]
        for g in range(2):
            nc.sync.dma_start(wcs_sb[g], wcs2[g, :, :])

        # ---- persistent activations ----
        qfin = [persist.tile([128, T], BF16, tag=f"qfin{t}", name=f"qfin{t}")
                for t in range(FT)]
        kfin = [persist.tile([128, T], BF16, tag=f"kfin{t}", name=f"kfin{t}")
                for t in range(FT)]
        vaug = [persist.tile([128, KB * (DH + 1)], BF16, tag=f"vaug{h}",
                             name=f"vaug{h}") for h in range(HPG)]
        ycore2 = [persist.tile([128, T], BF16, tag=f"yc{g}", name=f"yc{g}")
                  for g in range(FT)]

        # ones columns of vaug (written once) + ones row for den broadcast
        for h in range(HPG):
            va3 = vaug[h].rearrange("p (b c) -> p b c", c=DH + 1)
            nc.vector.memset(va3[:, :, DH], 1.0)
        ones1 = consts.tile([1, DH], BF16, tag="ones1", name="ones1")
        nc.vector.memset(ones1, 1.0)

        # ================= projection phase =================
        with tc.tile_pool(name="proj_ps", bufs=1, space="PSUM") as pps, \
             tc.tile_pool(name="proj_up_ps", bufs=1, space="PSUM") as ups, \
             tc.tile_pool(name="proj_sb", bufs=2) as psb:
            for t in range(TC):
                tsl = slice(t * 512, (t + 1) * 512)
                # down-projection: 6 accumulating banks (q0 q1 k0 k1 v0 v1)
                lat_ps = [pps.tile([128, 512], F32, tag=f"lat{i}", name=f"lat{i}")
                          for i in range(6)]
                xh = xA if t < 2 else xB
                xsl = slice((t % 2) * 512, (t % 2) * 512 + 512)
                for k in range(KC):
                    for wi in range(3):
                        for ft in range(FT):
                            nc.tensor.matmul(
                                lat_ps[2 * wi + ft],
                                lhsT=dw_sb[wi][:, k * 256 + ft * 128:
                                               k * 256 + (ft + 1) * 128],
                                rhs=xh[k][:, xsl],
                                start=(k == 0), stop=(k == KC - 1))
                # PSUM -> SBUF on the (idle) scalar engine
                lat_sb = [psb.tile([128, 512], BF16, tag=f"latsb{i}",
                                   name=f"latsb{i}") for i in range(4)]
                for i in range(4):
                    nc.scalar.copy(lat_sb[i], lat_ps[i])
                vlat_sb = [psb.tile([128, 512], BF16, tag=f"vlatsb{ft}",
                                    name=f"vlatsb{ft}") for ft in range(FT)]
                for ft in range(FT):
                    nc.scalar.copy(vlat_sb[ft], lat_ps[4 + ft])
                # up-projection + rope for q and k
                for fin, lats, co in ((qfin, lat_sb[0:2], 0),
                                      (kfin, lat_sb[2:4], 256)):
                    for ft in range(FT):
                        cep = ups.tile([128, 512], F32, tag="cep", name="cep")
                        nc.tensor.matmul(cep, lhsT=upw_sb[:, co:co + 128],
                                         rhs=lats[ft], start=True, stop=True)
                        rop = ups.tile([128, 512], F32, tag="rop", name="rop")
                        nc.tensor.matmul(rop, lhsT=upw_sb[:, co + 128:co + 256],
                                         rhs=lats[ft], start=True, stop=True)
                        tmp1 = psb.tile([128, 512], F32, tag="tmp1", name="tmp1")
                        tmp2 = psb.tile([128, 512], F32, tag="tmp2", name="tmp2")
                        nc.vector.tensor_mul(tmp1, cep, cos_sb[:, tsl])
                        nc.vector.tensor_mul(tmp2, rop, sin_sb[:, tsl])
                        nc.vector.tensor_add(fin[ft][:, tsl], tmp1, tmp2)
                # v up-projection for this chunk's 4 key blocks
                for ft in range(FT):
                    for off in (0, DH):
                        hsl = slice(off, off + DH)
                        h = 2 * ft + (1 if off else 0)
                        for blk in range(4):
                            ab = 4 * t + blk
                            vp = ups.tile([128, DH], F32, tag="cep", name="vp")
                            nc.tensor.matmul(
                                vp,
                                lhsT=vlat_sb[ft][hsl, blk * 128:(blk + 1) * 128],
                                rhs=vu2_sb[hsl, :], start=True, stop=True)
                            nc.vector.tensor_copy(
                                vaug[h][:, ab * 65:ab * 65 + DH], vp)

        # ================= attention + c_proj, query-chunk outer ========
        with tc.tile_pool(name="sc_ps", bufs=2, space="PSUM") as scp, \
             tc.tile_pool(name="yt_ps", bufs=3, space="PSUM") as ytp, \
             tc.tile_pool(name="out_ps", bufs=1, space="PSUM") as ops, \
             tc.tile_pool(name="att_sb", bufs=3) as asb, \
             tc.tile_pool(name="den_sb", bufs=4) as dsbp, \
             tc.tile_pool(name="rec_sb", bufs=2) as rp, \
             tc.tile_pool(name="out_sb", bufs=2) as osbp:
            for j in range(QB):
                qsl = slice(j * 512, (j + 1) * 512)
                for ft in range(FT):
                    # the two heads of this latent tile live on disjoint
                    # partition halves -> their K=64 score matmuls run
                    # row-tiled / concurrently on the PE array
                    hsls = [slice(0, DH), slice(DH, 128)]
                    yps = [ytp.tile([DH + 1, 512], F32, tag="yt", name="yt")
                           for _ in range(2)]

                    def score(hx, sp_sl, i, c0):
                        nc.tensor.matmul(
                            sp_sl,
                            lhsT=kfin[ft][hsls[hx], i * 128:(i + 1) * 128],
                            rhs=qfin[ft][hsls[hx],
                                         j * 512 + c0:(j + 1) * 512],
                            start=True, stop=True)

                    def yacc(hx, i, c0, pr_sl, start, stop):
                        nc.tensor.matmul(
                            yps[hx][:, c0:512],
                            lhsT=vaug[2 * ft + hx][:, i * 65:(i + 1) * 65],
                            rhs=pr_sl, start=start, stop=stop)

                    def block_pair(i0, c0a, wa, i1, c0b, wb, wexp,
                                   masks, start, stop):
                        sps, prs = [], []
                        for hx in range(2):
                            sp = scp.tile([128, 1024], F32, tag="sc",
                                          name="sc")
                            score(hx, sp[:, 0:wa], i0, c0a)
                            score(hx, sp[:, 512:512 + wb], i1, c0b)
                            sps.append(sp)
                        for hx in range(2):
                            pr = asb.tile([128, 1024], BF16, tag="pr",
                                          name="pr")
                            nc.scalar.activation(
                                pr[:, 0:wexp], sps[hx][:, 0:wexp],
                                mybir.ActivationFunctionType.Exp)
                            for mof in masks:
                                nc.vector.tensor_mul(
                                    pr[:, mof:mof + 128],
                                    pr[:, mof:mof + 128], mask_sb)
                            prs.append(pr)
                        for hx in range(2):
                            yacc(hx, i0, c0a, prs[hx][:, 0:wa], start, False)
                            yacc(hx, i1, c0b, prs[hx][:, 512:512 + wb],
                                 False, stop)

                    for a in range(2 * j):
                        block_pair(2 * a, 0, 512, 2 * a + 1, 0, 512, 1024,
                                   (), a == 0, False)
                    # diagonal pairs: d=0+d=1, then d=2+d=3 (exp spans the
                    # unused [256:512) gap of the second pair tile)
                    block_pair(4 * j, 0, 512, 4 * j + 1, 128, 384, 896,
                               (0, 512), j == 0, False)
                    block_pair(4 * j + 2, 256, 256, 4 * j + 3, 384, 128, 640,
                               (0, 512), False, True)
                    # softmax denominator: copy row (cast bf16), broadcast
                    # to 64 rows via a K=1 ones-matmul, then one DVE
                    # reciprocal over the whole [64, 512] block
                    for hx in range(2):
                        off = DH * hx
                        dsb = dsbp.tile([1, 512], BF16, tag="den", name="den")
                        nc.vector.tensor_copy(dsb, yps[hx][DH:DH + 1, :])
                        den64 = ytp.tile([DH, 512], F32, tag="yt",
                                         name="den64")
                        nc.tensor.matmul(den64, lhsT=ones1, rhs=dsb,
                                         start=True, stop=True)
                        rec64 = rp.tile([DH, 512], F32, tag="rec64",
                                        name="rec64")
                        if USE_FAST_RECIP:
                            nc.vector.reciprocal_approx_fast(out=rec64,
                                                             in_=den64)
                        else:
                            nc.vector.reciprocal(rec64, den64)
                        nc.vector.tensor_mul(ycore2[ft][off:off + DH, qsl],
                                             yps[hx][0:DH, :], rec64)
                # c_proj for this chunk's 4 row tiles
                for m in range(4 * j, 4 * j + 4):
                    msl = slice(m * 128, (m + 1) * 128)
                    osb = osbp.tile([128, C], BF16, tag="osb", name="osb")
                    for n in range(2):
                        nsl = slice(n * 512, (n + 1) * 512)
                        op = ops.tile([128, 512], F32, tag="op", name="op")
                        nc.tensor.matmul(op, lhsT=ycore2[0][:, msl],
                                         rhs=wcs_sb[0][:, nsl],
                                         start=True, stop=False)
                        nc.tensor.matmul(op, lhsT=ycore2[1][:, msl],
                                         rhs=wcs_sb[1][:, nsl],
                                         start=False, stop=True)
                        nc.vector.tensor_copy(osb[:, nsl], op)
                    nc.sync.dma_start(out[msl, :], osb)


def _host_prep(x, Wq_down, Wk_down, Wv_down, Wq_up_c, Wq_up_e, Wk_up_c,
               Wk_up_e, Wv_up, Wc):
    """Build the per-core input maps."""
    import math

    scale = 1.0 / math.sqrt(DH)

    # rope cache, transposed: (DHE, T)
    inv_freq = 1.0 / (THETA ** (np.arange(0, DHE, 2, dtype=np.float32) / DHE))
    freqs = np.arange(T, dtype=np.float32)[:, None] * inv_freq[None, :]
    emb = np.concatenate((freqs, freqs), axis=-1)  # (T, 32)
    cosT = np.cos(emb).T.astype(np.float32)  # (32, T)
    sinT = np.sin(emb).T.astype(np.float32)

    # signed permutation P: rot = P @ x with rot[2i] = -x[2i+1], rot[2i+1] = x[2i]
    P = np.zeros((DHE, DHE), dtype=np.float32)
    for i in range(DHE // 2):
        P[2 * i, 2 * i + 1] = -1.0
        P[2 * i + 1, 2 * i] = 1.0

    def ce_lhsT(Wc_, We_, s):
        # (128, 128): latents of 2 heads on partitions ->
        # [c_even | e_even | c_odd | e_odd] output rows
        m = np.zeros((128, 128), dtype=np.float32)
        m[0:64, 0:32] = Wc_ * s
        m[0:64, 32:64] = We_ * s
        m[64:128, 64:96] = Wc_ * s
        m[64:128, 96:128] = We_ * s
        return m

    def rot_lhsT(We_, s):
        m = np.zeros((128, 128), dtype=np.float32)
        wr = (We_ @ P.T) * s
        m[0:64, 32:64] = wr
        m[64:128, 96:128] = wr
        return m

    upw = np.concatenate(
        [ce_lhsT(Wq_up_c, Wq_up_e, scale), rot_lhsT(Wq_up_e, scale),
         ce_lhsT(Wk_up_c, Wk_up_e, 1.0), rot_lhsT(Wk_up_e, 1.0)], axis=1)
    vu2 = np.concatenate([Wv_up, Wv_up], axis=0).astype(np.float32)  # (128, 64)

    # cosM rows: [ones, cosT, ones, cosT]; sinM rows: [0, sinT, 0, sinT]
    ones = np.ones((32, T), dtype=np.float32)
    zeros = np.zeros((32, T), dtype=np.float32)
    cosM = np.concatenate([ones, cosT, ones, cosT], axis=0)
    sinM = np.concatenate([zeros, sinT, zeros, sinT], axis=0)

    kk = np.arange(128)[:, None]
    qq = np.arange(128)[None, :]
    mask128 = (kk <= qq).astype(np.float32)

    xTs = [np.ascontiguousarray(x[b].T).astype(np.float32) for b in range(B)]

    def pack_dw(W, csl):
        # (1024, 256) -> (128, 2048) with col = k*256 + ft*128 + l
        w = np.ascontiguousarray(W[:, csl])
        return np.ascontiguousarray(
            w.reshape(KC, 128, HPG * L).transpose(1, 0, 2).reshape(128, -1))

    bf = ml_dtypes.bfloat16
    in_maps = []
    for core in range(8):
        b, hg = core // HG, core % HG
        csl = slice(hg * HPG * L, (hg + 1) * HPG * L)
        in_maps.append({
            "xT": xTs[b].astype(bf),
            "dwq": pack_dw(Wq_down, csl).astype(bf),
            "dwk": pack_dw(Wk_down, csl).astype(bf),
            "dwv": pack_dw(Wv_down, csl).astype(bf),
            "upw": upw.astype(bf),
            "vu2": vu2.astype(bf),
            "cosM": cosM, "sinM": sinM,
            "mask128": mask128.astype(bf),
            "wcs2": np.ascontiguousarray(
                Wc[csl, :].reshape(2, 128, C)).astype(bf),
        })
    return in_maps


LAST_RESULT = {}


def kernel(**inputs):
    inputs = {k: np.asarray(v, dtype=np.float32) for k, v in inputs.items()}
    nc = _build_nc()
    in_maps = _host_prep(**inputs)
    res = run_bass_kernel_spmd(nc, in_maps, core_ids=list(range(8)))
    LAST_RESULT.clear()
    LAST_RESULT.update(
        exec_time_ns=res.exec_time_ns,
        mean_exec_time_ns=res.mean_exec_time_ns,
        profile_json=res.profile_json,
    )
    parts = [r["out"].astype(np.float32) for r in res.results]
    out = np.stack([
        parts[0] + parts[1] + parts[2] + parts[3],
        parts[4] + parts[5] + parts[6] + parts[7],
    ])
    return out.astype(np.float32)


if __name__ == "__main__":
    rng = np.random.default_rng(0)
    ins = {
        "x": rng.standard_normal((B, T, C), dtype=np.float32),
        "Wq_down": rng.standard_normal((C, H * L), dtype=np.float32) * 0.02,
        "Wk_down": rng.standard_normal((C, H * L), dtype=np.float32) * 0.02,
        "Wv_down": rng.standard_normal((C, H * L), dtype=np.float32) * 0.02,
        "Wq_up_c": rng.standard_normal((L, DHE), dtype=np.float32) * 0.02,
        "Wq_up_e": rng.standard_normal((L, DHE), dtype=np.float32) * 0.02,
        "Wk_up_c": rng.standard_normal((L, DHE), dtype=np.float32) * 0.02,
        "Wk_up_e": rng.standard_normal((L, DHE), dtype=np.float32) * 0.02,
        "Wv_up": rng.standard_normal((L, DH), dtype=np.float32) * 0.02,
        "Wc": rng.standard_normal((C, C), dtype=np.float32) * 0.02,
    }
    y = kernel(**ins)
    print(y.shape, y.dtype, float(np.abs(y).mean()))


# revision 3
# speedup vs baseline: 11740.7297x; 1.0143x over previous
"""MLA-style sparse-attention GPT block on 8 Trainium2 NeuronCores — v2.

Sharding: tensor-parallel over heads x data-parallel over batch.
Core c handles batch b = c // 4 and heads [4*hg, 4*hg+4) with hg = c % 4.
Each core computes its partial c_proj output (2048, 1024); the host sums
the 4 partials per batch.

v2 layout/scheduling changes vs v1:
- all inputs arrive in a handful of large DMAs (k-major packed weights),
  x is held resident in SBUF as 8 [128, 2048] tiles
- v up-projection + vaug assembly folded into the projection phase
- attention runs query-chunk-outer (j) with causal column restriction on
  the diagonal blocks; the output projection for chunk j's rows is
  interleaved right after chunk j's normalize, under the exp roof
- softmax reciprocal is a single DVE reciprocal_approx_fast on the
  denominator row; broadcast via a DRAM round-trip DMA
- ycore is packed as two [128, T] head-pair tiles so c_proj runs K=128
"""

import sys

sys.path.insert(0, "/opt/trn_rl_repo")

import ml_dtypes
import numpy as np

import concourse.bass as bass
import concourse.tile as tile
from concourse import bacc
from concourse import mybir
from concourse.bass_utils import run_bass_kernel_spmd

B, T, C = 2, 2048, 1024
H, L = 16, 64
DH = 64
DHE = 32
THETA = 10000.0

HG = 4  # head-groups (cores per batch)
HPG = H // HG  # heads per core = 4
FT = HPG // 2  # 2 latent tiles of 128 rows (2 heads each)

KC = C // 128  # 8 contraction chunks for the down-projection
TC = T // 512  # 4 chunks of 512 along T
QB = T // 512  # query chunks of 512
KB = T // 128  # key blocks of 128

F32 = mybir.dt.float32
BF16 = mybir.dt.bfloat16

USE_FAST_RECIP = True

_NC_CACHE = {}


def _build_nc():
    if "nc" in _NC_CACHE:
        return _NC_CACHE["nc"]
    nc = bacc.Bacc("TRN2", target_bir_lowering=False)

    xT = nc.dram_tensor("xT", [C, T], BF16, kind="ExternalInput")
    dwq = nc.dram_tensor("dwq", [128, KC * HPG * L // 1], BF16, kind="ExternalInput")
    dwk = nc.dram_tensor("dwk", [128, KC * HPG * L // 1], BF16, kind="ExternalInput")
    dwv = nc.dram_tensor("dwv", [128, KC * HPG * L // 1], BF16, kind="ExternalInput")
    upw = nc.dram_tensor("upw", [128, 512], BF16, kind="ExternalInput")
    vu2 = nc.dram_tensor("vu2", [128, DH], BF16, kind="ExternalInput")
    cosM = nc.dram_tensor("cosM", [128, T], F32, kind="ExternalInput")
    sinM = nc.dram_tensor("sinM", [128, T], F32, kind="ExternalInput")
    mask128 = nc.dram_tensor("mask128", [128, 128], BF16, kind="ExternalInput")
    wcs2 = nc.dram_tensor("wcs2", [2, 128, C], BF16, kind="ExternalInput")
    out = nc.dram_tensor("out", [T, C], BF16, kind="ExternalOutput")

    with tile.TileContext(nc) as tc:
        _emit(nc, tc, xT, dwq, dwk, dwv, upw, vu2, cosM, sinM, mask128,
              wcs2, out)
    nc.compile()

    _NC_CACHE["nc"] = nc
    return nc


def _emit(nc, tc, xT, dwq, dwk, dwv, upw, vu2, cosM, sinM, mask128, wcs2, out):
    from contextlib import ExitStack

    ctx = ExitStack()
    with ctx:
        consts = ctx.enter_context(tc.tile_pool(name="consts", bufs=1))
        persist = ctx.enter_context(tc.tile_pool(name="persist", bufs=1))

        # ---- batched input DMAs (order matters: first-needed first) ----
        # x held as two half-T tiles per k-chunk so chunk t<2 compute can
        # start before the back half of x arrives; all issues on the SP ring
        dwq_sb = consts.tile([128, KC * 256], BF16, tag="dwq", name="dwq")
        nc.sync.dma_start(dwq_sb, dwq[:, :])
        xA = [consts.tile([128, 1024], BF16, tag=f"xA{k}", name=f"xA{k}")
              for k in range(KC)]
        xB = [consts.tile([128, 1024], BF16, tag=f"xB{k}", name=f"xB{k}")
              for k in range(KC)]
        nc.sync.dma_start(xA[0], xT[0:128, 0:1024])
        dwk_sb = consts.tile([128, KC * 256], BF16, tag="dwk", name="dwk")
        nc.sync.dma_start(dwk_sb, dwk[:, :])
        dwv_sb = consts.tile([128, KC * 256], BF16, tag="dwv", name="dwv")
        nc.sync.dma_start(dwv_sb, dwv[:, :])
        dw_sb = [dwq_sb, dwk_sb, dwv_sb]
        vu2_sb = consts.tile([128, DH], BF16, tag="vu2", name="vu2")
        nc.sync.dma_start(vu2_sb, vu2[:, :])
        for k in (1, 2):
            nc.sync.dma_start(xA[k], xT[k * 128:(k + 1) * 128, 0:1024])
        upw_sb = consts.tile([128, 512], BF16, tag="upw", name="upw")
        nc.sync.dma_start(upw_sb, upw[:, :])
        cos_sb = consts.tile([128, T], F32, tag="cos", name="cos")
        sin_sb = consts.tile([128, T], F32, tag="sin", name="sin")
        nc.sync.dma_start(cos_sb, cosM[:, :])
        nc.sync.dma_start(sin_sb, sinM[:, :])
        for k in range(3, KC):
            nc.sync.dma_start(xA[k], xT[k * 128:(k + 1) * 128, 0:1024])
        mask_sb = consts.tile([128, 128], BF16, tag="mask", name="mask")
        nc.sync.dma_start(mask_sb, mask128[:, :])
        for k in range(KC):
            nc.sync.dma_start(xB[k], xT[k * 128:(k + 1) * 128, 1024:2048])
        wcs_sb = [consts.tile([128, C], BF16, tag=f"wcs{g}", name=f"wcs{g}")
                  for g in range(2)]
        for g in range(2):
            nc.sync.dma_start(wcs_sb[g], wcs2[g, :, :])

        # ---- persistent activations ----
        qfin = [persist.tile([128, T], BF16, tag=f"qfin{t}", name=f"qfin{t}")
                for t in range(FT)]
        kfin = [persist.tile([128, T], BF16, tag=f"kfin{t}", name=f"kfin{t}")
                for t in range(FT)]
        vaug = [persist.tile([128, KB * (DH + 1)], BF16, tag=f"vaug{h}",
                             name=f"vaug{h}") for h in range(HPG)]
        ycore2 = [persist.tile([128, T], BF16, tag=f"yc{g}", name=f"yc{g}")
                  for g in range(FT)]

        # ones columns of vaug (written once) + ones row for den broadcast
        for h in range(HPG):
            va3 = vaug[h].rearrange("p (b c) -> p b c", c=DH + 1)
            nc.vector.memset(va3[:, :, DH], 1.0)
        ones1 = consts.tile([1, DH], BF16, tag="ones1", name="ones1")
        nc.vector.memset(ones1, 1.0)

        # ================= projection phase =================
        with tc.tile_pool(name="proj_ps", bufs=1, space="PSUM") as pps, \
             tc.tile_pool(name="proj_up_ps", bufs=1, space="PSUM") as ups, \
             tc.tile_pool(name="proj_sb", bufs=2) as psb:
            for t in range(TC):
                tsl = slice(t * 512, (t + 1) * 512)
                # down-projection: 6 accumulating banks (q0 q1 k0 k1 v0 v1)
                lat_ps = [pps.tile([128, 512], F32, tag=f"lat{i}", name=f"lat{i}")
                          for i in range(6)]
                xh = xA if t < 2 else xB
                xsl = slice((t % 2) * 512, (t % 2) * 512 + 512)
                for k in range(KC):
                    for wi in range(3):
                        for ft in range(FT):
                            nc.tensor.matmul(
                                lat_ps[2 * wi + ft],
                                lhsT=dw_sb[wi][:, k * 256 + ft * 128:
                                               k * 256 + (ft + 1) * 128],
                                rhs=xh[k][:, xsl],
                                start=(k == 0), stop=(k == KC - 1))
                # PSUM -> SBUF on the (idle) scalar engine
                lat_sb = [psb.tile([128, 512], BF16, tag=f"latsb{i}",
                                   name=f"latsb{i}") for i in range(4)]
                for i in range(4):
                    nc.scalar.copy(lat_sb[i], lat_ps[i])
                vlat_sb = [psb.tile([128, 512], BF16, tag=f"vlatsb{ft}",
                                    name=f"vlatsb{ft}") for ft in range(FT)]
                for ft in range(FT):
                    nc.scalar.copy(vlat_sb[ft], lat_ps[4 + ft])
                # up-projection + rope for q and k
                for fin, lats, co in ((qfin, lat_sb[0:2], 0),
                                      (kfin, lat_sb[2:4], 256)):
                    for ft in range(FT):
                        cep = ups.tile([128, 512], F32, tag="cep", name="cep")
                        nc.tensor.matmul(cep, lhsT=upw_sb[:, co:co + 128],
                                         rhs=lats[ft], start=True, stop=True)
                        rop = ups.tile([128, 512], F32, tag="rop", name="rop")
                        nc.tensor.matmul(rop, lhsT=upw_sb[:, co + 128:co + 256],
                                         rhs=lats[ft], start=True, stop=True)
                        tmp1 = psb.tile([128, 512], F32, tag="tmp1", name="tmp1")
                        tmp2 = psb.tile([128, 512], F32, tag="tmp2", name="tmp2")
                        nc.vector.tensor_mul(tmp1, cep, cos_sb[:, tsl])
                        nc.vector.tensor_mul(tmp2, rop, sin_sb[:, tsl])
                        nc.vector.tensor_add(fin[ft][:, tsl], tmp1, tmp2)
                # v up-projection for this chunk's 4 key blocks
                for ft in range(FT):
                    for off in (0, DH):
                        hsl = slice(off, off + DH)
                        h = 2 * ft + (1 if off else 0)
                        for blk in range(4):
                            ab = 4 * t + blk
                            vp = ups.tile([128, DH], F32, tag="cep", name="vp")
                            nc.tensor.matmul(
                                vp,
                                lhsT=vlat_sb[ft][hsl, blk * 128:(blk + 1) * 128],
                                rhs=vu2_sb[hsl, :], start=True, stop=True)
                            nc.vector.tensor_copy(
                                vaug[h][:, ab * 65:ab * 65 + DH], vp)

        # ================= attention + c_proj, query-chunk outer ========
        with tc.tile_pool(name="sc_ps", bufs=2, space="PSUM") as scp, \
             tc.tile_pool(name="yt_ps", bufs=3, space="PSUM") as ytp, \
             tc.tile_pool(name="out_ps", bufs=1, space="PSUM") as ops, \
             tc.tile_pool(name="att_sb", bufs=4) as asb, \
             tc.tile_pool(name="den_sb", bufs=4) as dsbp, \
             tc.tile_pool(name="rec_sb", bufs=3) as rp, \
             tc.tile_pool(name="out_sb", bufs=3) as osbp:
            for j in range(QB):
                qsl = slice(j * 512, (j + 1) * 512)
                for ft in range(FT):
                    # the two heads of this latent tile live on disjoint
                    # partition halves -> their K=64 score matmuls run
                    # row-tiled / concurrently on the PE array
                    hsls = [slice(0, DH), slice(DH, 128)]
                    yps = [ytp.tile([DH + 1, 512], F32, tag="yt", name="yt")
                           for _ in range(2)]

                    def score(hx, sp_sl, i, c0):
                        nc.tensor.matmul(
                            sp_sl,
                            lhsT=kfin[ft][hsls[hx], i * 128:(i + 1) * 128],
                            rhs=qfin[ft][hsls[hx],
                                         j * 512 + c0:(j + 1) * 512],
                            start=True, stop=True)

                    def yacc(hx, i, c0, pr_sl, start, stop):
                        nc.tensor.matmul(
                            yps[hx][:, c0:512],
                            lhsT=vaug[2 * ft + hx][:, i * 65:(i + 1) * 65],
                            rhs=pr_sl, start=start, stop=stop)

                    def block_pair(i0, c0a, wa, i1, c0b, wb, wexp,
                                   masks, start, stop):
                        sps, prs = [], []
                        for hx in range(2):
                            sp = scp.tile([128, 1024], F32, tag="sc",
                                          name="sc")
                            score(hx, sp[:, 0:wa], i0, c0a)
                            score(hx, sp[:, 512:512 + wb], i1, c0b)
                            sps.append(sp)
                        for hx in range(2):
                            pr = asb.tile([128, 1024], BF16, tag="pr",
                                          name="pr")
                            nc.scalar.activation(
                                pr[:, 0:wexp], sps[hx][:, 0:wexp],
                                mybir.ActivationFunctionType.Exp)
                            for mof in masks:
                                nc.vector.tensor_mul(
                                    pr[:, mof:mof + 128],
                                    pr[:, mof:mof + 128], mask_sb)
                            prs.append(pr)
                        for hx in range(2):
                            yacc(hx, i0, c0a, prs[hx][:, 0:wa], start, False)
                            yacc(hx, i1, c0b, prs[hx][:, 512:512 + wb],
                                 False, stop)

                    for a in range(2 * j):
                        block_pair(2 * a, 0, 512, 2 * a + 1, 0, 512, 1024,
                                   (), a == 0, False)
                    # diagonal pairs: d=0+d=1, then d=2+d=3 (exp spans the
                    # unused [256:512) gap of the second pair tile)
                    block_pair(4 * j, 0, 512, 4 * j + 1, 128, 384, 896,
                               (0, 512), j == 0, False)
                    block_pair(4 * j + 2, 256, 256, 4 * j + 3, 384, 128, 640,
                               (0, 512), False, True)
                    # softmax denominator: copy row (cast bf16), broadcast
                    # to 64 rows via a K=1 ones-matmul, then one DVE
                    # reciprocal over the whole [64, 512] block
                    for hx in range(2):
                        off = DH * hx
                        dsb = dsbp.tile([1, 512], BF16, tag="den", name="den")
                        nc.vector.tensor_copy(dsb, yps[hx][DH:DH + 1, :])
                        den64 = ytp.tile([DH, 512], F32, tag="yt",
                                         name="den64")
                        nc.tensor.matmul(den64, lhsT=ones1, rhs=dsb,
                                         start=True, stop=True)
                        rec64 = rp.tile([DH, 512], F32, tag="rec64",
                                        name="rec64")
                        if USE_FAST_RECIP:
                            nc.vector.reciprocal_approx_fast(out=rec64,
                                                             in_=den64)
                        else:
                            nc.vector.reciprocal(rec64, den64)
                        nc.vector.tensor_mul(ycore2[ft][off:off + DH, qsl],
                                             yps[hx][0:DH, :], rec64)
                # c_proj for this chunk's 4 row tiles
                for m in range(4 * j, 4 * j + 4):
                    msl = slice(m * 128, (m + 1) * 128)
                    osb = osbp.tile([128, C], BF16, tag="osb", name="osb")
                    for n in range(2):
                        nsl = slice(n * 512, (n + 1) * 512)
                        op = ops.tile([128, 512], F32, tag="op", name="op")
                        nc.tensor.matmul(op, lhsT=ycore2[0][:, msl],
                                         rhs=wcs_sb[0][:, nsl],
                                         start=True, stop=False)
                        nc.tensor.matmul(op, lhsT=ycore2[1][:, msl],
                                         rhs=wcs_sb[1][:, nsl],
                                         start=False, stop=True)
                        nc.vector.tensor_copy(osb[:, nsl], op)
                    nc.sync.dma_start(out[msl, :], osb)


def _host_prep(x, Wq_down, Wk_down, Wv_down, Wq_up_c, Wq_up_e, Wk_up_c,
               Wk_up_e, Wv_up, Wc):
    """Build the per-core input maps."""
    import math

    scale = 1.0 / math.sqrt(DH)

    # rope cache, transposed: (DHE, T)
    inv_freq = 1.0 / (THETA ** (np.arange(0, DHE, 2, dtype=np.float32) / DHE))
    freqs = np.arange(T, dtype=np.float32)[:, None] * inv_freq[None, :]
    emb = np.concatenate((freqs, freqs), axis=-1)  # (T, 32)
    cosT = np.cos(emb).T.astype(np.float32)  # (32, T)
    sinT = np.sin(emb).T.astype(np.float32)

    # signed permutation P: rot = P @ x with rot[2i] = -x[2i+1], rot[2i+1] = x[2i]
    P = np.zeros((DHE, DHE), dtype=np.float32)
    for i in range(DHE // 2):
        P[2 * i, 2 * i + 1] = -1.0
        P[2 * i + 1, 2 * i] = 1.0

    def ce_lhsT(Wc_, We_, s):
        # (128, 128): latents of 2 heads on partitions ->
        # [c_even | e_even | c_odd | e_odd] output rows
        m = np.zeros((128, 128), dtype=np.float32)
        m[0:64, 0:32] = Wc_ * s
        m[0:64, 32:64] = We_ * s
        m[64:128, 64:96] = Wc_ * s
        m[64:128, 96:128] = We_ * s
        return m

    def rot_lhsT(We_, s):
        m = np.zeros((128, 128), dtype=np.float32)
        wr = (We_ @ P.T) * s
        m[0:64, 32:64] = wr
        m[64:128, 96:128] = wr
        return m

    upw = np.concatenate(
        [ce_lhsT(Wq_up_c, Wq_up_e, scale), rot_lhsT(Wq_up_e, scale),
         ce_lhsT(Wk_up_c, Wk_up_e, 1.0), rot_lhsT(Wk_up_e, 1.0)], axis=1)
    vu2 = np.concatenate([Wv_up, Wv_up], axis=0).astype(np.float32)  # (128, 64)

    # cosM rows: [ones, cosT, ones, cosT]; sinM rows: [0, sinT, 0, sinT]
    ones = np.ones((32, T), dtype=np.float32)
    zeros = np.zeros((32, T), dtype=np.float32)
    cosM = np.concatenate([ones, cosT, ones, cosT], axis=0)
    sinM = np.concatenate([zeros, sinT, zeros, sinT], axis=0)

    kk = np.arange(128)[:, None]
    qq = np.arange(128)[None, :]
    mask128 = (kk <= qq).astype(np.float32)

    xTs = [np.ascontiguousarray(x[b].T).astype(np.float32) for b in range(B)]

    def pack_dw(W, csl):
        # (1024, 256) -> (128, 2048) with col = k*256 + ft*128 + l
        w = np.ascontiguousarray(W[:, csl])
        return np.ascontiguousarray(
            w.reshape(KC, 128, HPG * L).transpose(1, 0, 2).reshape(128, -1))

    bf = ml_dtypes.bfloat16
    in_maps = []
    for core in range(8):
        b, hg = core // HG, core % HG
        csl = slice(hg * HPG * L, (hg + 1) * HPG * L)
        in_maps.append({
            "xT": xTs[b].astype(bf),
            "dwq": pack_dw(Wq_down, csl).astype(bf),
            "dwk": pack_dw(Wk_down, csl).astype(bf),
            "dwv": pack_dw(Wv_down, csl).astype(bf),
            "upw": upw.astype(bf),
            "vu2": vu2.astype(bf),
            "cosM": cosM, "sinM": sinM,
            "mask128": mask128.astype(bf),
            "wcs2": np.ascontiguousarray(
                Wc[csl, :].reshape(2, 128, C)).astype(bf),
        })
    return in_maps


LAST_RESULT = {}


def kernel(**inputs):
    inputs = {k: np.asarray(v, dtype=np.float32) for k, v in inputs.items()}
    nc = _build_nc()
    in_maps = _host_prep(**inputs)
    res = run_bass_kernel_spmd(nc, in_maps, core_ids=list(range(8)))
    LAST_RESULT.clear()
    LAST_RESULT.update(
        exec_time_ns=res.exec_time_ns,
        mean_exec_time_ns=res.mean_exec_time_ns,
        profile_json=res.profile_json,
    )
    parts = [r["out"].astype(np.float32) for r in res.results]
    out = np.stack([
        parts[0] + parts[1] + parts[2] + parts[3],
        parts[4] + parts[5] + parts[6] + parts[7],
    ])
    return out.astype(np.float32)


if __name__ == "__main__":
    rng = np.random.default_rng(0)
    ins = {
        "x": rng.standard_normal((B, T, C), dtype=np.float32),
        "Wq_down": rng.standard_normal((C, H * L), dtype=np.float32) * 0.02,
        "Wk_down": rng.standard_normal((C, H * L), dtype=np.float32) * 0.02,
        "Wv_down": rng.standard_normal((C, H * L), dtype=np.float32) * 0.02,
        "Wq_up_c": rng.standard_normal((L, DHE), dtype=np.float32) * 0.02,
        "Wq_up_e": rng.standard_normal((L, DHE), dtype=np.float32) * 0.02,
        "Wk_up_c": rng.standard_normal((L, DHE), dtype=np.float32) * 0.02,
        "Wk_up_e": rng.standard_normal((L, DHE), dtype=np.float32) * 0.02,
        "Wv_up": rng.standard_normal((L, DH), dtype=np.float32) * 0.02,
        "Wc": rng.standard_normal((C, C), dtype=np.float32) * 0.02,
    }
    y = kernel(**ins)
    print(y.shape, y.dtype, float(np.abs(y).mean()))
